# revision 8
# baseline (speedup 1.0000x reference)
"""Trainium2 Bass kernel for nn_Detector (YOLO-style detector decode).

Contract: kernel(**inputs) takes the FULL unsharded inputs from
setup_inputs() and returns the FULL [340704, 90] fp32 output. The batch
dim (32) is sharded across 8 NeuronCores (4 images per core).

Design (v2, fp16 I/O):
  The decode is pure elementwise work at ~32 MB/core of fp32 HBM traffic,
  so the kernel is DMA-bound. Two structural changes vs the fp32 version:

  1. The host pre-transposes each image into row-major [128, 90, comp]
     chunk layout (hw = chunk*128 + partition), so the device needs no
     TensorEngine transposes and no PSUM at all -- engines only do the
     decode math on [128, 90-group, k] slices.

  2. I/O is fp16, halving HBM bytes. Precision hazards are handled
     surgically: p/dx/dy stay f32 in a small side tensor (exact mask
     compare; no (ix+dx) cancellation), and the point/seg-coord channels
     are pre-scaled x256 so neither the fp16 inputs nor the fp16 outputs
     land in the denormal range (the host divides the matching output
     columns by 256 after upcasting). Measured vs the f32 reference:
     fro rel err ~3e-4, elementwise rel max ~2e-3, zero mask flips.

  Per image: one 1.9MB fp16 load, ~24 ACT/DVE ops, one 2.1MB fp16 store.
  sigmoid(x) = 0.5*tanh(x/2)+0.5 keeps tanh+exp in one ScalarE table set
  (exp_and_others); sqrt is batched per image-pair so the set switch
  costs 2x2.7us per pair instead of per image. The affine+mask
  (0.5m*tanh + 0.5m) fuses into one scalar_tensor_tensor DVE op.
"""
import numpy as np

f32np = np.float32
f16np = np.float16

B = 32
N_CORES = 8
B_LOCAL = B // N_CORES

# g-groups are scale-major: hw = c*128 + p, g = goff + c*3 + a
# (name, W, t, HW, T, goff)
SCALES = [("52", 52, 8.0, 2704, 22, 0),
          ("26", 26, 16.0, 676, 6, 66),
          ("13", 13, 32.0, 169, 2, 84)]
G = 90          # total groups = 3*(22+6+2)
NCOMP_H = 86    # fp16 comps: dw,dh | point*12 (x256) | segc*24 (x256) | segl*48
NCOMP_F = 3     # f32 comps: p, dx, dy
SC = 256.0      # denormal-avoidance pre-scale on point/seg-coord channels

# device-side output column order keeps the seg coords/logits contiguous
# (every engine op stays <=3D and the big multiplies fuse); the host
# permutes columns back to the interleaved reference order at unpack:
# dev: [n, sig, cx, cy, w, h, point*12, segcoord*24, seglogit*48]

# consts column layout [128, NC] f32
_THRL = 0                            # 1 col: logit(thresh)
_NTAB = 1                            # 4 cols: n per local image
_AW = {"52": 5, "26": 11, "13": 17}  # 6 cols each: (aw,ah) per anchor
_IXY = {"52": 23, "26": 67, "13": 79}  # 2T cols each: (ix,iy) per chunk
NC = 83

_CACHE = {}


def _build_nc(case):
    import concourse.bacc as bacc
    import concourse.tile as tile
    from concourse import mybir

    f32 = mybir.dt.float32
    f16 = mybir.dt.float16
    AF = mybir.ActivationFunctionType
    OP = mybir.AluOpType

    nc = bacc.Bacc("TRN2", target_bir_lowering=False, debug=False)
    xh = nc.declare_dram_parameter("xh", [B_LOCAL, 128, G, NCOMP_H], f16,
                                   isOutput=False)
    xf = nc.declare_dram_parameter("xf", [128, B_LOCAL, G, NCOMP_F], f32,
                                   isOutput=False)
    consts = nc.declare_dram_parameter("consts", [128, NC], f32,
                                       isOutput=False)
    y = nc.declare_dram_parameter("y", [B_LOCAL, 128, G, 90], f16,
                                  isOutput=True)

    with tile.TileContext(nc) as tc:
        with (
            tc.tile_pool(name="single", bufs=1) as single,
            tc.tile_pool(name="inp", bufs=3) as in_pool,
            tc.tile_pool(name="outp", bufs=3) as out_pool,
            tc.tile_pool(name="small", bufs=3) as small,
        ):
            ct = single.tile([128, NC], f32)
            nc.sync.dma_start(out=ct[:], in_=consts[:])
            xft = single.tile([128, B_LOCAL, G, NCOMP_F], f32)
            nc.sync.dma_start(out=xft[:], in_=xf[:])

            state = {}

            def phase_load(b):
                in_h = in_pool.tile([128, G, NCOMP_H], f16, tag="inh")
                nc.sync.dma_start(out=in_h[:], in_=xh[b])
                out_t = out_pool.tile([128, G, 90], f16, tag="out")
                state[b] = [in_h, out_t]

            def phase_exp(b):
                # [exp_and_others set] wh = exp(dw,dh)
                in_h, out_t = state[b]
                wh = small.tile([128, G, 2], f32, tag="wh")
                nc.scalar.activation(wh[:], in_h[:, :, 0:2], AF.Exp)
                state[b].append(wh)

            def phase_sig(b):
                # [sigmoid_and_others set] objectness + seg sigmoids,
                # written at full value (no tanh affine -> no fp16
                # cancellation for small sigmoid outputs)
                in_h, out_t, wh = state[b]
                sg = small.tile([128, G], f32, tag="sg")
                nc.scalar.activation(sg[:], xft[:, b, :, 0], AF.Sigmoid)
                nc.scalar.activation(out_t[:, :, 42:90], in_h[:, :, 38:86],
                                     AF.Sigmoid)
                state[b].append(sg)

            def phase_dve(b):
                in_h, out_t, wh, sg = state[b]
                p_ap = xft[:, b, :, 0]
                m = small.tile([128, G], f32, tag="m")
                nc.vector.tensor_scalar(m[:], p_ap, ct[:, _THRL:_THRL + 1],
                                        None, op0=OP.is_gt)
                # col 1: sigmoid(p)*m ; col 0: n*m
                nc.vector.tensor_mul(out_t[:, :, 1], sg[:], m[:])
                nc.vector.tensor_scalar(
                    out_t[:, :, 0], m[:], ct[:, _NTAB + b:_NTAB + b + 1],
                    None, op0=OP.mult)
                # cols 4,5: w,h = anchors * exp
                for name, W, t, HW, T, goff in SCALES:
                    awo = _AW[name]
                    whs = wh[:, goff:goff + 3 * T].rearrange(
                        "p (c a) k -> p c a k", a=3)
                    nc.vector.tensor_mul(
                        whs, whs,
                        ct[:, awo:awo + 6].rearrange(
                            "p (a k) -> p a k", k=2).unsqueeze(1).broadcast_to(
                                (128, T, 3, 2)))
                sq = small.tile([128, G, 2], f32, tag="sq")
                nc.vector.tensor_mul(sq[:], wh[:], wh[:])
                q = small.tile([128, G], f32, tag="q")
                nc.vector.tensor_add(q[:], sq[:, :, 0], sq[:, :, 1])
                # cols 2,3: (dx+ix, dy+iy) * t * m
                t2 = small.tile([128, G, 2], f32, tag="t2")
                for name, W, t, HW, T, goff in SCALES:
                    ixo = _IXY[name]
                    nc.vector.tensor_add(
                        t2[:, goff:goff + 3 * T].rearrange(
                            "p (c a) k -> p c a k", a=3),
                        xft[:, b, goff:goff + 3 * T, 1:3].rearrange(
                            "p (c a) k -> p c a k", a=3),
                        ct[:, ixo:ixo + 2 * T].rearrange(
                            "p (c k) -> p c k", k=2).unsqueeze(2).broadcast_to(
                                (128, T, 3, 2)))
                mt = small.tile([128, G], f32, tag="mt")
                for name, W, t, HW, T, goff in SCALES:
                    nc.vector.tensor_scalar_mul(
                        mt[:, goff:goff + 3 * T], m[:, goff:goff + 3 * T],
                        float(t))
                nc.vector.tensor_mul(
                    out_t[:, :, 2:4], t2[:],
                    mt[:].unsqueeze(2).broadcast_to((128, G, 2)))
                nc.vector.tensor_mul(
                    out_t[:, :, 4:6], wh[:],
                    m[:].unsqueeze(2).broadcast_to((128, G, 2)))
                # seg sigmoids * m (in place on dev cols 42:90)
                nc.vector.tensor_mul(
                    out_t[:, :, 42:90], out_t[:, :, 42:90],
                    m[:].unsqueeze(2).broadcast_to((128, G, 48)))
                state[b] = [in_h, out_t, m, q]

            def phase_s(b):
                # [sqrt_and_others set] batched per pair
                _, _, _, q = state[b]
                s = small.tile([128, G], f32, tag="s")
                nc.scalar.activation(s[:], q[:], AF.Sqrt,
                                     scale=1.0 / (float(case) * float(case)))
                state[b].append(s)

            def phase_d(b):
                in_h, out_t, m, q, s = state[b]
                ms = small.tile([128, G], f32, tag="ms")
                nc.vector.tensor_mul(ms[:], s[:], m[:])
                # point (dev cols 6:18) + seg coords (dev cols 18:42) fused
                nc.vector.tensor_mul(
                    out_t[:, :, 6:42], in_h[:, :, 2:38],
                    ms[:].unsqueeze(2).broadcast_to((128, G, 36)))
                nc.sync.dma_start(out=y[b], in_=out_t[:])

            for pair in range(B_LOCAL // 2):
                bs = (2 * pair, 2 * pair + 1)
                for b in bs:
                    phase_load(b)
                for b in bs:
                    phase_exp(b)
                for b in bs:
                    phase_sig(b)
                for b in bs:
                    phase_dve(b)
                for b in bs:
                    phase_s(b)
                for b in bs:
                    phase_d(b)
    nc.compile()
    return nc


# fp16 channel selection: index in original 90-comp input vector, in the
# device order [dw, dh, point*12, segcoord*24, seglogit*48]
_CHI = ([3, 4] + list(range(6, 18)) + list(range(18, 90, 3))
        + [c for k in range(24) for c in (19 + 3 * k, 20 + 3 * k)])
_CSCL = np.ones(NCOMP_H, f32np)
_CSCL[2:14] = SC   # point
_CSCL[14:38] = SC  # seg coords

# host unpack: final output column <- device column
_SRC = np.empty(90, np.int64)
_SRC[0:6] = np.arange(0, 6)
_SRC[6:18] = np.arange(6, 18)
_SRC[18:90:3] = 18 + np.arange(24)
_SRC[19:90:3] = 42 + 2 * np.arange(24)
_SRC[20:90:3] = 43 + 2 * np.arange(24)


def _host_consts(core, anchors, thr_logit):
    ct = np.zeros((128, NC), f32np)
    ct[:, _THRL] = thr_logit
    for b in range(B_LOCAL):
        ct[:, _NTAB + b] = f32np(core * B_LOCAL + b)
    for name, W, t, HW, T, goff in SCALES:
        a = anchors[name].astype(f32np)  # [3, 2]
        ct[:, _AW[name]:_AW[name] + 6] = a.reshape(-1)[None, :]
        hw = np.arange(T)[None, :] * 128 + np.arange(128)[:, None]  # [128,T]
        o = _IXY[name]
        ct[:, o:o + 2 * T:2] = (hw % W).astype(f32np)
        ct[:, o + 1:o + 2 * T:2] = (hw // W).astype(f32np)
    return ct


def _pack_inputs(out13, out26, out52, anchors, thresh):
    xs = {"13": np.asarray(out13, f32np), "26": np.asarray(out26, f32np),
          "52": np.asarray(out52, f32np)}
    xh = np.empty((B, 128, G, NCOMP_H), f16np)
    xf = np.empty((B, 128, B_LOCAL, G, NCOMP_F), f32np)  # sliced per core
    # xf is really [128, B_LOCAL, G, 3] per core; build [B,128,G,3] then remap
    xf_b = np.empty((B, 128, G, NCOMP_F), f32np)
    for name, W, t, HW, T, goff in SCALES:
        v = xs[name].reshape(B, 3, 90, HW)
        arr = np.zeros((B, 3, 90, T * 128), f32np)
        arr[..., :HW] = v
        # [B, 3, 90, T, 128] -> [B, 128, T, 3, 90]
        arr = arr.reshape(B, 3, 90, T, 128).transpose(0, 4, 3, 1, 2)
        blk = arr.reshape(B, 128, 3 * T, 90)
        xh[:, :, goff:goff + 3 * T, :] = (
            blk[..., _CHI] * _CSCL).astype(f16np)
        xf_b[:, :, goff:goff + 3 * T, :] = blk[..., 0:3]

    thr = np.float64(np.asarray(thresh, f32np)[0])
    thr_logit = f32np(np.log(thr / (1.0 - thr)))
    in_maps = []
    for core in range(N_CORES):
        bs = slice(core * B_LOCAL, (core + 1) * B_LOCAL)
        m = {
            "xh": np.ascontiguousarray(xh[bs]),
            # [B_LOCAL,128,G,3] -> [128,B_LOCAL,G,3]
            "xf": np.ascontiguousarray(xf_b[bs].transpose(1, 0, 2, 3)),
            "consts": _host_consts(core, anchors, thr_logit),
        }
        in_maps.append(m)
    return in_maps


def _unpack_outputs(res):
    rows = {name: B * HW * 3 for name, _, _, HW, _, _ in SCALES}
    out = np.empty((rows["13"] + rows["26"] + rows["52"], 90), f32np)
    region = {"13": 0, "26": rows["13"], "52": rows["13"] + rows["26"]}
    for core in range(N_CORES):
        yv = res[core]["y"]  # [B_LOCAL, 128, G, 90] fp16, device col order
        for name, W, t, HW, T, goff in SCALES:
            # [B_LOCAL, 128, T, 3, 90] -> [B_LOCAL, T, 128, 3, 90]
            arr = yv[:, :, goff:goff + 3 * T, :].reshape(
                B_LOCAL, 128, T, 3, 90).transpose(0, 2, 1, 3, 4)
            arr = arr.reshape(B_LOCAL, T * 128, 3, 90)[:, :HW]
            n = B_LOCAL * HW * 3
            out[region[name] + core * n:region[name] + (core + 1) * n] = \
                arr.reshape(n, 90)[:, _SRC].astype(f32np)
    out[:, 6:18] *= f32np(1.0 / SC)
    out[:, 18:90:3] *= f32np(1.0 / SC)
    return out


def kernel(out13, out26, out52, anchors13, anchors26, anchors52, thresh,
           case, **kw):
    from concourse.bass_utils import run_bass_kernel_spmd

    anchors = {"13": np.asarray(anchors13), "26": np.asarray(anchors26),
               "52": np.asarray(anchors52)}
    key = ("nc", int(case))
    if key not in _CACHE:
        _CACHE[key] = _build_nc(int(case))
    nc = _CACHE[key]

    in_maps = _pack_inputs(out13, out26, out52, anchors,
                           np.asarray(thresh, f32np))
    res = run_bass_kernel_spmd(nc, in_maps, list(range(N_CORES))).results
    return _unpack_outputs(res)


# revision 9
# speedup vs baseline: 1.0318x; 1.0318x over previous
"""Trainium2 Bass kernel for nn_Detector (YOLO-style detector decode).

Contract: kernel(**inputs) takes the FULL unsharded inputs from
setup_inputs() and returns the FULL [340704, 90] fp32 output. The batch
dim (32) is sharded across 8 NeuronCores (4 images per core).

Design (v3, fp16 I/O, no PE/PSUM):
  The decode is pure elementwise work, so the kernel is DMA-bound. The
  host pre-transposes each image into row-major [128, group, comp] chunk
  layout (hw = chunk*128 + partition) and ships fp16, halving HBM bytes;
  the device does only the decode math -- no TensorEngine, no PSUM.

  Precision (validated against the f32 reference on the real data:
  fro ~3e-4, elementwise rel max ~2e-3, zero mask flips):
  - p/dx/dy stay f32 in a small side tensor: exact threshold compare on
    raw p (vs logit(thresh)), and no (ix+dx) cancellation in fp16.
  - point/seg-coord channels are pre-scaled x256 so neither fp16 input
    nor fp16 output hits the denormal range; the host divides those
    output columns by 256 after upcasting.
  - seg sigmoids use AF.Sigmoid directly (a tanh+affine form would round
    tanh~-1 through fp16 and amplify into the small sigmoid outputs).

  Engine budget per core (4 images): one fp16 load + one fp16 store per
  image (plus consts/xf once); ~14 DVE ops and 5 ACT ops per image, all
  full-image [128, 90, k] slices. ACT table sets are pinned via explicit
  same-engine deps to the rotation [sigmoid] -> [natural_log_exp] per
  image pair (sqrt is computed as exp(0.5*ln(q)) to stay in one set with
  exp). Device output columns keep seg coords/logits contiguous; the
  host permutes columns back to the interleaved reference order.
"""
import numpy as np

f32np = np.float32
f16np = np.float16

B = 32
N_CORES = 8
B_LOCAL = B // N_CORES

# g-groups are scale-major: hw = c*128 + p, g = goff + c*3 + a
# (name, W, t, HW, T, goff)
SCALES = [("52", 52, 8.0, 2704, 22, 0),
          ("26", 26, 16.0, 676, 6, 66),
          ("13", 13, 32.0, 169, 2, 84)]
G = 90          # total groups = 3*(22+6+2)
NCOMP_H = 86    # fp16 comps: dw,dh | point*12 (x256) | segc*24 (x256) | segl*48
NCOMP_F = 3     # f32 comps: p, t*dx, t*dy   (t is a power of 2 -> exact)
SC = 256.0      # denormal-avoidance pre-scale on point/seg-coord channels

# consts column layout [128, NC] f32
_THRL = 0                 # 1 col: logit(thresh)
_NTAB = 1                 # 4 cols: n per local image
_AWF = 5                  # 180 cols: (aw,ah)[g] full table
_IXYT = 185               # 180 cols: (t*ix, t*iy)[g] full table
NC = 365

_CACHE = {}


def _build_nc(case):
    import concourse.bacc as bacc
    import concourse.tile as tile
    from concourse import mybir
    from concourse.tile_rust import add_dep_helper

    f32 = mybir.dt.float32
    f16 = mybir.dt.float16
    AF = mybir.ActivationFunctionType
    OP = mybir.AluOpType

    nc = bacc.Bacc("TRN2", target_bir_lowering=False, debug=False)
    xh = nc.declare_dram_parameter("xh", [B_LOCAL, 128, G, NCOMP_H], f16,
                                   isOutput=False)
    xf = nc.declare_dram_parameter("xf", [128, B_LOCAL, G, NCOMP_F], f32,
                                   isOutput=False)
    consts = nc.declare_dram_parameter("consts", [128, NC], f32,
                                       isOutput=False)
    y = nc.declare_dram_parameter("y", [B_LOCAL, 128, G, 90], f16,
                                  isOutput=True)

    last_act = [None]

    def act(*args, **kw):
        # pin ScalarE program order so the table-set rotation holds
        ins = nc.scalar.activation(*args, **kw)
        if last_act[0] is not None:
            add_dep_helper(ins.ins, last_act[0].ins, sync=True,
                           reason="act table-set order")
        last_act[0] = ins
        return ins

    with tile.TileContext(nc) as tc:
        with (
            tc.tile_pool(name="single", bufs=1) as single,
            tc.tile_pool(name="inp", bufs=4) as in_pool,
            tc.tile_pool(name="outp", bufs=4) as out_pool,
            tc.tile_pool(name="small", bufs=4) as small,
        ):
            ct = single.tile([128, NC], f32)
            nc.sync.dma_start(out=ct[:], in_=consts[:])
            xft = single.tile([128, B_LOCAL, G, NCOMP_F], f32)
            nc.sync.dma_start(out=xft[:], in_=xf[:])

            state = {}

            def phase_load(b):
                in_h = in_pool.tile([128, G, NCOMP_H], f16, tag="inh")
                nc.sync.dma_start(out=in_h[:], in_=xh[b])
                out_t = out_pool.tile([128, G, 90], f16, tag="out")
                state[b] = [in_h, out_t]

            def phase_sg(b):
                # [sigmoid set] objectness sigmoid (from f32 p)
                sg = small.tile([128, G], f32, tag="sg")
                act(sg[:], xft[:, b, :, 0], AF.Sigmoid)
                state[b].append(sg)

            def phase_segs(b):
                # [sigmoid set] seg sigmoids -> dev cols 42:90 (fp16)
                in_h, out_t, sg = state[b]
                act(out_t[:, :, 42:90], in_h[:, :, 38:86], AF.Sigmoid)

            def phase_exp(b):
                # [natural_log_exp set] wh = exp(dw,dh)
                in_h, out_t, sg = state[b]
                wh = small.tile([128, G, 2], f32, tag="wh")
                act(wh[:], in_h[:, :, 0:2], AF.Exp)
                state[b].append(wh)

            def phase_dve(b):
                in_h, out_t, sg, wh = state[b]
                m = small.tile([128, G], f32, tag="m")
                nc.vector.tensor_scalar(m[:], xft[:, b, :, 0],
                                        ct[:, _THRL:_THRL + 1], None,
                                        op0=OP.is_gt)
                m16 = small.tile([128, G], f16, tag="m16")
                nc.vector.tensor_copy(m16[:], m[:])
                # col 1: sigmoid(p)*m ; col 0: n*m
                nc.vector.tensor_mul(out_t[:, :, 1], sg[:], m[:])
                nc.vector.tensor_scalar(
                    out_t[:, :, 0], m[:], ct[:, _NTAB + b:_NTAB + b + 1],
                    None, op0=OP.mult)
                # cols 4,5: w,h = anchors * exp  (in place on wh, f32)
                nc.vector.tensor_mul(
                    wh[:], wh[:],
                    ct[:, _AWF:_AWF + 180].rearrange(
                        "p (g k) -> p g k", k=2))
                sq = small.tile([128, G, 2], f32, tag="sq")
                nc.vector.tensor_mul(sq[:], wh[:], wh[:])
                q = small.tile([128, G], f32, tag="q")
                nc.vector.tensor_add(q[:], sq[:, :, 0], sq[:, :, 1])
                # cols 2,3: (t*dx + t*ix) * m
                t2 = small.tile([128, G, 2], f32, tag="t2")
                nc.vector.tensor_add(
                    t2[:], xft[:, b, :, 1:3],
                    ct[:, _IXYT:_IXYT + 180].rearrange(
                        "p (g k) -> p g k", k=2))
                nc.vector.tensor_mul(
                    out_t[:, :, 2:4], t2[:],
                    m[:].unsqueeze(2).broadcast_to((128, G, 2)))
                nc.vector.tensor_mul(
                    out_t[:, :, 4:6], wh[:],
                    m[:].unsqueeze(2).broadcast_to((128, G, 2)))
                # seg sigmoids * m (pure fp16)
                nc.vector.tensor_mul(
                    out_t[:, :, 42:90], out_t[:, :, 42:90],
                    m16[:].unsqueeze(2).broadcast_to((128, G, 48)))
                state[b] = [in_h, out_t, m, q]

            def phase_s(b):
                # [natural_log_exp set] s = sqrt(q)/case = exp(0.5*ln(q))
                _, _, _, q = state[b]
                u = small.tile([128, G], f32, tag="u")
                act(u[:], q[:], AF.Ln,
                    scale=1.0 / (float(case) * float(case)))
                s = small.tile([128, G], f32, tag="s")
                act(s[:], u[:], AF.Exp, scale=0.5)
                state[b].append(s)

            def phase_d(b):
                in_h, out_t, m, q, s = state[b]
                ms = small.tile([128, G], f32, tag="ms")
                nc.vector.tensor_mul(ms[:], s[:], m[:])
                ms16 = small.tile([128, G], f16, tag="ms16")
                nc.vector.tensor_copy(ms16[:], ms[:])
                # point (dev cols 6:18) + seg coords (18:42), pure fp16
                nc.vector.tensor_mul(
                    out_t[:, :, 6:42], in_h[:, :, 2:38],
                    ms16[:].unsqueeze(2).broadcast_to((128, G, 36)))
                nc.sync.dma_start(out=y[b], in_=out_t[:])

            for b in range(B_LOCAL):
                phase_load(b)
            for b in range(B_LOCAL):
                phase_sg(b)          # [sigmoid] x4, one table load
            for pair in range(B_LOCAL // 2):
                bs = (2 * pair, 2 * pair + 1)
                for b in bs:
                    phase_segs(b)    # [sigmoid]
                for b in bs:
                    phase_exp(b)     # [natural_log_exp]
                for b in bs:
                    phase_dve(b)
                for b in bs:
                    phase_s(b)       # [natural_log_exp]
                for b in bs:
                    phase_d(b)
    nc.compile()
    return nc


# fp16 channel selection: index in original 90-comp input vector, in the
# device order [dw, dh, point*12, segcoord*24, seglogit*48]
_CHI = ([3, 4] + list(range(6, 18)) + list(range(18, 90, 3))
        + [c for k in range(24) for c in (19 + 3 * k, 20 + 3 * k)])
_CSCL = np.ones(NCOMP_H, f32np)
_CSCL[2:14] = SC   # point
_CSCL[14:38] = SC  # seg coords

# host unpack: final output column <- device column
_SRC = np.empty(90, np.int64)
_SRC[0:6] = np.arange(0, 6)
_SRC[6:18] = np.arange(6, 18)
_SRC[18:90:3] = 18 + np.arange(24)
_SRC[19:90:3] = 42 + 2 * np.arange(24)
_SRC[20:90:3] = 43 + 2 * np.arange(24)


def _host_consts(core, anchors, thr_logit):
    ct = np.zeros((128, NC), f32np)
    ct[:, _THRL] = thr_logit
    for b in range(B_LOCAL):
        ct[:, _NTAB + b] = f32np(core * B_LOCAL + b)
    for name, W, t, HW, T, goff in SCALES:
        a = anchors[name].astype(f32np).reshape(-1)  # aw0,ah0,aw1,ah1,aw2,ah2
        # full (aw,ah)[g] table: g = c*3 + anchor
        ct[:, _AWF + 2 * goff:_AWF + 2 * (goff + 3 * T)] = np.tile(
            a[None, :], (128, T))
        hw = np.arange(T)[None, :] * 128 + np.arange(128)[:, None]  # [128,T]
        ix = (hw % W).astype(f32np) * f32np(t)
        iy = (hw // W).astype(f32np) * f32np(t)
        o = _IXYT + 2 * goff
        # per g = (c, a): same (ix,iy) for all 3 anchors of a chunk
        blk = np.empty((128, T, 3, 2), f32np)
        blk[:, :, :, 0] = ix[:, :, None]
        blk[:, :, :, 1] = iy[:, :, None]
        ct[:, o:o + 6 * T] = blk.reshape(128, 6 * T)
    return ct


def _pack_inputs(out13, out26, out52, anchors, thresh):
    xs = {"13": np.asarray(out13, f32np), "26": np.asarray(out26, f32np),
          "52": np.asarray(out52, f32np)}
    xh = np.empty((B, 128, G, NCOMP_H), f16np)
    xf_b = np.empty((B, 128, G, NCOMP_F), f32np)
    for name, W, t, HW, T, goff in SCALES:
        v = xs[name].reshape(B, 3, 90, HW)
        arr = np.zeros((B, 3, 90, T * 128), f32np)
        arr[..., :HW] = v
        # [B, 3, 90, T, 128] -> [B, 128, T, 3, 90]
        arr = arr.reshape(B, 3, 90, T, 128).transpose(0, 4, 3, 1, 2)
        blk = arr.reshape(B, 128, 3 * T, 90)
        xh[:, :, goff:goff + 3 * T, :] = (
            blk[..., _CHI] * _CSCL).astype(f16np)
        xf_b[:, :, goff:goff + 3 * T, 0] = blk[..., 0]
        xf_b[:, :, goff:goff + 3 * T, 1:3] = blk[..., 1:3] * f32np(t)

    thr = np.float64(np.asarray(thresh, f32np)[0])
    thr_logit = f32np(np.log(thr / (1.0 - thr)))
    in_maps = []
    for core in range(N_CORES):
        bs = slice(core * B_LOCAL, (core + 1) * B_LOCAL)
        m = {
            "xh": np.ascontiguousarray(xh[bs]),
            # [B_LOCAL,128,G,3] -> [128,B_LOCAL,G,3]
            "xf": np.ascontiguousarray(xf_b[bs].transpose(1, 0, 2, 3)),
            "consts": _host_consts(core, anchors, thr_logit),
        }
        in_maps.append(m)
    return in_maps


def _unpack_outputs(res):
    rows = {name: B * HW * 3 for name, _, _, HW, _, _ in SCALES}
    out = np.empty((rows["13"] + rows["26"] + rows["52"], 90), f32np)
    region = {"13": 0, "26": rows["13"], "52": rows["13"] + rows["26"]}
    for core in range(N_CORES):
        yv = res[core]["y"]  # [B_LOCAL, 128, G, 90] fp16, device col order
        for name, W, t, HW, T, goff in SCALES:
            # [B_LOCAL, 128, T, 3, 90] -> [B_LOCAL, T, 128, 3, 90]
            arr = yv[:, :, goff:goff + 3 * T, :].reshape(
                B_LOCAL, 128, T, 3, 90).transpose(0, 2, 1, 3, 4)
            arr = arr.reshape(B_LOCAL, T * 128, 3, 90)[:, :HW]
            n = B_LOCAL * HW * 3
            out[region[name] + core * n:region[name] + (core + 1) * n] = \
                arr.reshape(n, 90)[:, _SRC].astype(f32np)
    out[:, 6:18] *= f32np(1.0 / SC)
    out[:, 18:90:3] *= f32np(1.0 / SC)
    return out


def kernel(out13, out26, out52, anchors13, anchors26, anchors52, thresh,
           case, **kw):
    from concourse.bass_utils import run_bass_kernel_spmd

    anchors = {"13": np.asarray(anchors13), "26": np.asarray(anchors26),
               "52": np.asarray(anchors52)}
    key = ("nc", int(case))
    if key not in _CACHE:
        _CACHE[key] = _build_nc(int(case))
    nc = _CACHE[key]

    in_maps = _pack_inputs(out13, out26, out52, anchors,
                           np.asarray(thresh, f32np))
    res = run_bass_kernel_spmd(nc, in_maps, list(range(N_CORES))).results
    return _unpack_outputs(res)


# revision 10
# speedup vs baseline: 1.3837x; 1.3412x over previous
"""Trainium2 Bass kernel for nn_Detector (YOLO-style detector decode).

Contract: kernel(**inputs) takes the FULL unsharded inputs from
setup_inputs() and returns the FULL [340704, 90] fp32 output. The batch
dim (32) is sharded across 8 NeuronCores (4 images per core).

Design (v4, fp16 I/O, comp-major, no PE/PSUM):
  The decode is pure elementwise work, so the kernel is DMA-bound. The
  host pre-transposes each image into chunk layout (hw = c*128 +
  partition) and ships fp16, halving HBM bytes; the device does only
  decode math -- no TensorEngine, no PSUM.

  On-device tensors are COMP-MAJOR [128, comp, g] (g = 90 chunk-anchor
  groups innermost). The mask/scale broadcasts then sit on the outer
  free dim with a step-1 fp16 inner dim, which is what the DVE packed
  2x mode requires (a stride-0 inner dim drops to 1 elem/cycle).

  Precision (validated on the real data: fro ~3e-4, elementwise rel max
  ~2e-3, zero mask flips):
  - p/dx/dy stay f32 in a small side tensor: exact threshold compare on
    raw p vs logit(thresh); no (ix+dx) cancellation in fp16. dx/dy and
    the grid tables are pre-scaled by the stride t (a power of 2, exact).
  - point/seg-coord channels are pre-scaled x256 so neither fp16 input
    nor fp16 output hits the denormal range; the host divides those
    output columns by 256 after upcasting.
  - seg sigmoids use AF.Sigmoid directly (a tanh+affine form would
    round tanh~-1 through fp16 and amplify into small sigmoid outputs).

  ScalarE program order is pinned with explicit deps so the activation
  table-set rotation is [sigmoid] -> [exp] -> [sqrt] once per image
  pair (6 ACT_TABLE_LOADs total; the scheduler otherwise shuffles ACT
  ops and doubles the loads). Outputs are split into two DRAM tensors
  so the masked sigmoid block stores as soon as it is ready, before the
  sqrt/scale chain finishes the coord block.
"""
import numpy as np

f32np = np.float32
f16np = np.float16

B = 32
N_CORES = 8
B_LOCAL = B // N_CORES

# g-groups are scale-major: hw = c*128 + p, g = goff + c*3 + a
# (name, W, t, HW, T, goff)
SCALES = [("52", 52, 8.0, 2704, 22, 0),
          ("26", 26, 16.0, 676, 6, 66),
          ("13", 13, 32.0, 169, 2, 84)]
G = 90          # total groups = 3*(22+6+2)
NCOMP_H = 86    # fp16 comps: dw,dh | point*12 (x256) | segc*24 (x256) | segl*48
NCOMP_F = 3     # f32 comps: p, t*dx, t*dy
N_REST = 42     # out block 1: n, sig, cx, cy, w, h, point*12, segc*24
N_SIG = 48      # out block 2: seg logits -> sigmoids
SC = 256.0      # denormal-avoidance pre-scale on point/seg-coord channels

# consts column layout [128, NC] f32
_THRL = 0                 # 1 col: logit(thresh)
_NTAB = 1                 # 4 cols: n per local image
_AWF = 5                  # 180 cols: (aw,ah) as [2, 90] comp-major table
_IXYT = 185               # 180 cols: (t*ix, t*iy) as [2, 90] table
NC = 365

_CACHE = {}


def _build_nc(case):
    import concourse.bacc as bacc
    import concourse.tile as tile
    from concourse import mybir
    from concourse.tile_rust import add_dep_helper

    f32 = mybir.dt.float32
    f16 = mybir.dt.float16
    AF = mybir.ActivationFunctionType
    OP = mybir.AluOpType

    nc = bacc.Bacc("TRN2", target_bir_lowering=False, debug=False)
    xh = nc.declare_dram_parameter("xh", [B_LOCAL, 128, NCOMP_H, G], f16,
                                   isOutput=False)
    xf = nc.declare_dram_parameter("xf", [128, B_LOCAL, NCOMP_F, G], f32,
                                   isOutput=False)
    consts = nc.declare_dram_parameter("consts", [128, NC], f32,
                                       isOutput=False)
    y1 = nc.declare_dram_parameter("y1", [B_LOCAL, 128, N_REST, G], f16,
                                   isOutput=True)
    y2 = nc.declare_dram_parameter("y2", [B_LOCAL, 128, N_SIG, G], f16,
                                   isOutput=True)

    last_act = [None]

    def act(*args, **kw):
        # pin ScalarE program order so the table-set rotation holds
        ins = nc.scalar.activation(*args, **kw)
        if last_act[0] is not None:
            add_dep_helper(ins.ins, last_act[0].ins, sync=True,
                           reason="act table-set order")
        last_act[0] = ins
        return ins

    with tile.TileContext(nc) as tc:
        with (
            tc.tile_pool(name="single", bufs=1) as single,
            tc.tile_pool(name="inp", bufs=4) as in_pool,
            tc.tile_pool(name="outp", bufs=4) as out_pool,
            tc.tile_pool(name="small", bufs=4) as small,
        ):
            state = {}

            def phase_load(b):
                in_h = in_pool.tile([128, NCOMP_H, G], f16, tag="inh")
                nc.sync.dma_start(out=in_h[:], in_=xh[b])
                state[b] = [in_h]

            # first image load leads; consts/xf ride behind it
            phase_load(0)
            ct = single.tile([128, NC], f32)
            nc.sync.dma_start(out=ct[:], in_=consts[:])
            xft = single.tile([128, B_LOCAL, NCOMP_F, G], f32)
            nc.sync.dma_start(out=xft[:], in_=xf[:])
            for b in range(1, B_LOCAL):
                phase_load(b)

            awf = ct[:, _AWF:_AWF + 180].rearrange("p (k g) -> p k g", g=G)
            ixyt = ct[:, _IXYT:_IXYT + 180].rearrange("p (k g) -> p k g", g=G)

            def phase_sg(b):
                # [sigmoid set] objectness sigmoid (from f32 p)
                sg = small.tile([128, G], f32, tag="sg")
                act(sg[:], xft[:, b, 0, :], AF.Sigmoid)
                state[b].append(sg)

            def phase_segs(b):
                # [sigmoid set] seg sigmoids, full value, fp16 out
                in_h, sg = state[b]
                o_sig = out_pool.tile([128, N_SIG, G], f16, tag="osig")
                act(o_sig[:], in_h[:, 38:86, :], AF.Sigmoid)
                state[b].append(o_sig)

            def phase_exp(b):
                # [exp set] wh = exp(dw,dh)
                in_h, sg, o_sig = state[b]
                wh = small.tile([128, 2, G], f32, tag="wh")
                act(wh[:], in_h[:, 0:2, :], AF.Exp)
                state[b].append(wh)

            def phase_dve(b):
                in_h, sg, o_sig, wh = state[b]
                o_rest = out_pool.tile([128, N_REST, G], f16, tag="orest")
                m = small.tile([128, G], f32, tag="m")
                nc.vector.tensor_scalar(m[:], xft[:, b, 0, :],
                                        ct[:, _THRL:_THRL + 1], None,
                                        op0=OP.is_gt)
                m16 = small.tile([128, G], f16, tag="m16")
                nc.vector.tensor_copy(m16[:], m[:])
                # row 1: sigmoid(p)*m ; row 0: n*m
                nc.vector.tensor_mul(o_rest[:, 1, :], sg[:], m[:])
                nc.vector.tensor_scalar(
                    o_rest[:, 0, :], m[:], ct[:, _NTAB + b:_NTAB + b + 1],
                    None, op0=OP.mult)
                # rows 4,5: w,h = anchors * exp (in place on wh, f32)
                nc.vector.tensor_mul(wh[:], wh[:], awf)
                sq = small.tile([128, 2, G], f32, tag="sq")
                nc.vector.tensor_mul(sq[:], wh[:], wh[:])
                q = small.tile([128, G], f32, tag="q")
                nc.vector.tensor_add(q[:], sq[:, 0, :], sq[:, 1, :])
                # rows 2,3: (t*dx + t*ix) * m
                t2 = small.tile([128, 2, G], f32, tag="t2")
                nc.vector.tensor_add(t2[:], xft[:, b, 1:3, :], ixyt)
                nc.vector.tensor_mul(
                    o_rest[:, 2:4, :], t2[:],
                    m[:].unsqueeze(1).broadcast_to((128, 2, G)))
                nc.vector.tensor_mul(
                    o_rest[:, 4:6, :], wh[:],
                    m[:].unsqueeze(1).broadcast_to((128, 2, G)))
                # seg sigmoids * m: fp16 x fp16, step-1 inner -> packed 2x
                nc.vector.tensor_mul(
                    o_sig[:], o_sig[:],
                    m16[:].unsqueeze(1).broadcast_to((128, N_SIG, G)))
                nc.sync.dma_start(out=y2[b], in_=o_sig[:])
                state[b] = [in_h, o_rest, m, q]

            def phase_s(b):
                # [sqrt set] s = sqrt(q)/case, batched per pair
                _, _, _, q = state[b]
                s = small.tile([128, G], f32, tag="s")
                act(s[:], q[:], AF.Sqrt,
                    scale=1.0 / (float(case) * float(case)))
                state[b].append(s)

            def phase_d(b):
                in_h, o_rest, m, q, s = state[b]
                ms = small.tile([128, G], f32, tag="ms")
                nc.vector.tensor_mul(ms[:], s[:], m[:])
                ms16 = small.tile([128, G], f16, tag="ms16")
                nc.vector.tensor_copy(ms16[:], ms[:])
                # point + seg coords: fp16 x fp16 packed 2x
                nc.vector.tensor_mul(
                    o_rest[:, 6:42, :], in_h[:, 2:38, :],
                    ms16[:].unsqueeze(1).broadcast_to((128, 36, G)))
                nc.sync.dma_start(out=y1[b], in_=o_rest[:])

            for b in range(B_LOCAL):
                phase_sg(b)          # [sigmoid] x4, one table load
            for pair in range(B_LOCAL // 2):
                bs = (2 * pair, 2 * pair + 1)
                for b in bs:
                    phase_segs(b)    # [sigmoid]
                for b in bs:
                    phase_exp(b)     # [exp]
                for b in bs:
                    phase_dve(b)     # + y2 store
                for b in bs:
                    phase_s(b)       # [sqrt]
                for b in bs:
                    phase_d(b)       # + y1 store
    nc.compile()
    return nc


# fp16 channel selection: index in original 90-comp input vector, in the
# device order [dw, dh, point*12, segcoord*24, seglogit*48]
_CHI = ([3, 4] + list(range(6, 18)) + list(range(18, 90, 3))
        + [c for k in range(24) for c in (19 + 3 * k, 20 + 3 * k)])
_CSCL = np.ones(NCOMP_H, f32np)
_CSCL[2:14] = SC   # point
_CSCL[14:38] = SC  # seg coords

# host unpack: final output column <- device comp index (y1 ++ y2)
_SRC = np.empty(90, np.int64)
_SRC[0:6] = np.arange(0, 6)
_SRC[6:18] = np.arange(6, 18)
_SRC[18:90:3] = 18 + np.arange(24)
_SRC[19:90:3] = 42 + 2 * np.arange(24)
_SRC[20:90:3] = 43 + 2 * np.arange(24)


def _host_consts(core, anchors, thr_logit):
    ct = np.zeros((128, NC), f32np)
    ct[:, _THRL] = thr_logit
    for b in range(B_LOCAL):
        ct[:, _NTAB + b] = f32np(core * B_LOCAL + b)
    awf = np.empty((2, G), f32np)
    ixyt = np.empty((2, G), f32np)
    for name, W, t, HW, T, goff in SCALES:
        a = anchors[name].astype(f32np)  # [3, 2] = (aw, ah) per anchor
        # g = goff + c*3 + anchor
        awf[0, goff:goff + 3 * T] = np.tile(a[:, 0], T)
        awf[1, goff:goff + 3 * T] = np.tile(a[:, 1], T)
        hw = np.arange(T) * 128  # chunk base; ix/iy vary per partition
        # per-partition values: fill later (they depend on p)
    # ixyt depends on the partition -> build full [128, 2, G]
    ixyt_f = np.empty((128, 2, G), f32np)
    for name, W, t, HW, T, goff in SCALES:
        hw = np.arange(T)[None, :] * 128 + np.arange(128)[:, None]  # [128,T]
        ix = (hw % W).astype(f32np) * f32np(t)
        iy = (hw // W).astype(f32np) * f32np(t)
        ixyt_f[:, 0, goff:goff + 3 * T] = np.repeat(ix, 3, axis=1)
        ixyt_f[:, 1, goff:goff + 3 * T] = np.repeat(iy, 3, axis=1)
    ct[:, _AWF:_AWF + 180] = np.broadcast_to(
        awf.reshape(-1)[None, :], (128, 180))
    ct[:, _IXYT:_IXYT + 180] = ixyt_f.reshape(128, 180)
    return ct


def _pack_inputs(out13, out26, out52, anchors, thresh):
    xs = {"13": np.asarray(out13, f32np), "26": np.asarray(out26, f32np),
          "52": np.asarray(out52, f32np)}
    xh = np.empty((B, 128, NCOMP_H, G), f16np)
    xf_b = np.empty((B, 128, NCOMP_F, G), f32np)
    for name, W, t, HW, T, goff in SCALES:
        v = xs[name].reshape(B, 3, 90, HW)
        arr = np.zeros((B, 3, 90, T * 128), f32np)
        arr[..., :HW] = v
        # [B, 3, 90, T, 128] -> [B, 128, 90, T, 3] -> [B, 128, 90, 3T]
        arr = arr.reshape(B, 3, 90, T, 128).transpose(0, 4, 2, 3, 1)
        blk = arr.reshape(B, 128, 90, 3 * T)
        xh[:, :, :, goff:goff + 3 * T] = (
            blk[:, :, _CHI, :] * _CSCL[None, None, :, None]).astype(f16np)
        xf_b[:, :, 0, goff:goff + 3 * T] = blk[:, :, 0, :]
        xf_b[:, :, 1:3, goff:goff + 3 * T] = blk[:, :, 1:3, :] * f32np(t)

    thr = np.float64(np.asarray(thresh, f32np)[0])
    thr_logit = f32np(np.log(thr / (1.0 - thr)))
    in_maps = []
    for core in range(N_CORES):
        bs = slice(core * B_LOCAL, (core + 1) * B_LOCAL)
        m = {
            "xh": np.ascontiguousarray(xh[bs]),
            # [B_LOCAL,128,3,G] -> [128,B_LOCAL,3,G]
            "xf": np.ascontiguousarray(xf_b[bs].transpose(1, 0, 2, 3)),
            "consts": _host_consts(core, anchors, thr_logit),
        }
        in_maps.append(m)
    return in_maps


def _unpack_outputs(res):
    rows = {name: B * HW * 3 for name, _, _, HW, _, _ in SCALES}
    out = np.empty((rows["13"] + rows["26"] + rows["52"], 90), f32np)
    region = {"13": 0, "26": rows["13"], "52": rows["13"] + rows["26"]}
    for core in range(N_CORES):
        # [B_LOCAL, 128, 90, G] fp16, device comp order
        yv = np.concatenate([res[core]["y1"], res[core]["y2"]], axis=2)
        for name, W, t, HW, T, goff in SCALES:
            # [B_LOCAL, 128, 90, T, 3] -> [B_LOCAL, T, 128, 3, 90]
            arr = yv[:, :, :, goff:goff + 3 * T].reshape(
                B_LOCAL, 128, 90, T, 3).transpose(0, 3, 1, 4, 2)
            arr = arr.reshape(B_LOCAL, T * 128, 3, 90)[:, :HW]
            n = B_LOCAL * HW * 3
            out[region[name] + core * n:region[name] + (core + 1) * n] = \
                arr.reshape(n, 90)[:, _SRC].astype(f32np)
    out[:, 6:18] *= f32np(1.0 / SC)
    out[:, 18:90:3] *= f32np(1.0 / SC)
    return out


def kernel(out13, out26, out52, anchors13, anchors26, anchors52, thresh,
           case, **kw):
    from concourse.bass_utils import run_bass_kernel_spmd

    anchors = {"13": np.asarray(anchors13), "26": np.asarray(anchors26),
               "52": np.asarray(anchors52)}
    key = ("nc", int(case))
    if key not in _CACHE:
        _CACHE[key] = _build_nc(int(case))
    nc = _CACHE[key]

    in_maps = _pack_inputs(out13, out26, out52, anchors,
                           np.asarray(thresh, f32np))
    res = run_bass_kernel_spmd(nc, in_maps, list(range(N_CORES))).results
    return _unpack_outputs(res)
